# revision 78
# baseline (speedup 1.0000x reference)
"""Trainium2 Bass kernel for nn_Diffusion_ViTCA_NCA (ViT-Cellular-Automata, 6 NCA steps).

Data-parallel over batch B=8 across 8 NeuronCores (1 sample/core), fully
SBUF-resident state, feature-major half-stacked layout
[128 partitions = (2 row-halves x 64 channels), 2048 tokens-per-half].

Optimizations over the original kernel:
- perception fc0 folded into the 3x3 conv on the host (exact fp32 weight
  precompute, one fused 128-wide conv, no intermediate round trip)
- conv row-offset pairing: [half; half+1row] bf16 stacks (SBUF shift DMAs,
  split by row range) let two row-offsets share one 128-contraction matmul;
  plus column-shift pairing (xc = [half+1row; half+1row+1col]) for the
  di=+1 taps -- 5 matmuls per 512-token group instead of 6
- both image halves fused per matmul via block-diagonal weights
- ch0 freeze via zeroed weight columns (fc1t/outwbd/ff2bd rows for
  out-channel 0) instead of a restore pass
- norm0 global stats free-ride on activation accum_out; fc1 mean-correction
  via per-partition scalar; LN 1/sqrt via one ACT Abs_reciprocal_sqrt
- fused softmax-Z: one matmul (zrcpbl) sums the 9 taps AND broadcasts Z
  per head straight off Ebuf
- dots(k=1) software-pipelined against eb(k=0); dedicated pdot PSUM pool
  so the live dots accumulator never blocks the eb tile cycling; conv
  spreads its 4 PSUM tiles over p1k/pdot/pacc so no matmul waits evictions
- activation-table prefetch: tiny anchored dummy activations load the
  abs_rsqrt/exp/gelu tables during engine-idle windows (table load is
  1283ns and there are 4 switches per step)
- engine-balanced elementwise respecting real HW limits (Pool/GPSIMD has no
  PSUM port, no scalar_tensor_tensor; DVE 2x bf16 tensor_tensor mode),
  1024-col PSUM evictions, per-row-half splits for overlap, bulk mask DMA
  on the ACT DGE so SP stays free for the shift DMAs
"""

import numpy as np
import ml_dtypes

C = 64; HID = 128; HEADS = 4; HD = 16; H = 64; W = 64; B = 8; STEPS = 6
MLP = 64; SCALE = HD ** -0.5
RH = 32            # image rows per half
PR = 34            # padded rows per half buffer
PITCH = 68         # padded col pitch
NH = RH * W        # 2048 tokens per half
NPAD = PR * PITCH  # 2312
EPS = 1e-5
BF16 = ml_dtypes.bfloat16

_BUILD_CACHE = {}


def _pad_state(xb):
    """xb [H,W,C] -> padded half-stacked state [128, PR*PITCH] fp32."""
    cf = np.transpose(np.asarray(xb, np.float32), (2, 0, 1))   # [C,H,W]
    buf = np.zeros((128, PR, PITCH), np.float32)
    for s in range(2):
        buf[64 * s:64 * s + 64, 1:33, 2:66] = cf[:, RH * s:RH * s + RH, :]
    buf[:, 1:33, 1] = buf[:, 1:33, 3]
    buf[:, 1:33, 66] = buf[:, 1:33, 64]
    buf[0:64, 0, :] = buf[0:64, 2, :]
    buf[64:128, 33, :] = buf[64:128, 31, :]
    buf[0:64, 33, :] = buf[64:128, 1, :]
    buf[64:128, 0, :] = buf[0:64, 32, :]
    return buf.reshape(128, NPAD)


def _bd(a):
    """[64,64] -> [128,128] block-diagonal (same block twice)."""
    z = np.zeros((128, 128), np.float32)
    z[0:64, 0:64] = a
    z[64:128, 64:128] = a
    return z


def _pack_weights(ip):
    f32 = np.float32
    w = {}
    assert np.allclose(np.asarray(ip['norm0_w'], f32), 1.0), "norm0_w!=1 unsupported"
    assert np.allclose(np.asarray(ip['norm0_b'], f32), 0.0), "norm0_b!=0 unsupported"
    for _bn in ('p0_b', 'p1_b', 'fc0_b', 'ff1_b', 'ln1_b', 'ln2_b'):
        assert np.allclose(np.asarray(ip[_bn], f32), 0.0), _bn + "!=0 unsupported"

    # perception conv with fc0 folded in:
    #   hd = relu(sum_o W'_o^T x_shift_o),  W'_o [64in, 128hid]
    #   W'_o = cw_o @ fc0y  (+ fc0x for the center tap)
    p0, p1 = np.asarray(ip['p0_w'], f32), np.asarray(ip['p1_w'], f32)
    fc0 = np.asarray(ip['fc0_w'], f32)
    fc0x = fc0[:, 0:64].T                    # [64in, 128hid]
    fc0y = fc0[:, 64:192].T                  # [128y, 128hid]
    cw2 = np.zeros((64, 9, 128), f32)
    for i in range(3):
        for j in range(3):
            o = 3 * i + j
            cw_o = np.concatenate([p0[:, :, i, j].T, p1[:, :, i, j].T], 1)  # [64in,128y]
            wp = cw_o @ fc0y
            if o == 4:
                wp = wp + fc0x
            cw2[:, o, :] = wp
    # paired weights: offsets (di=-1,dj)+(di=0,dj) share one 128-contraction
    # matmul against [half; half+1row] (xb0) / [half+1row; half] (xb1) stacks
    cp0 = np.zeros((128, 3, 128), f32)
    cp1 = np.zeros((128, 3, 128), f32)
    cs_ = np.zeros((128, 3, 128), f32)
    for j in range(3):
        cp0[0:64, j] = cw2[:, 0 + j]      # di=-1 hits the unshifted top
        cp0[64:128, j] = cw2[:, 3 + j]    # di=0 hits the +1-row bottom
        cp1[0:64, j] = cw2[:, 3 + j]      # xb1: top is +1-row
        cp1[64:128, j] = cw2[:, 0 + j]
        cs_[0:64, j] = cw2[:, 6 + j]      # di=+1 singles, half0 (xb0 top)
        cs_[64:128, j] = cw2[:, 6 + j]    # half1 (xb1 bottom)
    w['convp0'] = cp0.reshape(128, 3 * 128).astype(BF16)
    w['convp1'] = cp1.reshape(128, 3 * 128).astype(BF16)
    w['convs'] = cs_.reshape(128, 3 * 128).astype(BF16)
    # column-shift pairing for di=+1: xc = [half+1row; half+1row+1col]
    # covers (di=+1,j=0) on top and (di=+1,j=1) on bottom in one matmul
    cc = np.zeros((128, 128), f32)
    cc[0:64] = cw2[:, 6]
    cc[64:128] = cw2[:, 7]
    w['convc'] = cc.astype(BF16)

    # ch0 freeze: the reference keeps input channel 0 unchanged each step.
    # Zero every weight column/bias that writes out-channel 0 so the three
    # residual adds never touch partitions 0/64 -- no restore pass needed.
    fc1 = np.asarray(ip['fc1_w'], f32).copy()
    fc1[0, :] = 0.0
    w['fc1t'] = fc1.T.copy().astype(BF16)
    rs = fc1.sum(1)
    w['fc1rs_col'] = np.concatenate([rs, rs]).reshape(128, 1).astype(f32)

    ln1w = np.asarray(ip['ln1_w'], f32)
    qkv = np.asarray(ip['qkv_w'], f32)
    qkv_eff = qkv * ln1w[None, :]
    qkv_eff[0:64] *= SCALE
    qt = qkv_eff.T                                     # [64in, 192]
    qkvbd = np.zeros((128, 3 * 128), f32)
    for m in range(3):
        qkvbd[:, 128 * m:128 * m + 128] = _bd(qt[:, 64 * m:64 * m + 64])
    w['qkvbd'] = qkvbd.astype(BF16)

    out_w = np.asarray(ip['out_w'], f32).copy()
    out_w[0, :] = 0.0
    w['outwbd'] = _bd(out_w.T).astype(BF16)
    outb = np.asarray(ip['out_b'], f32).copy()
    outb[0] = 0.0
    w['outb'] = np.concatenate([outb, outb]).reshape(128, 1)

    ln2w = np.asarray(ip['ln2_w'], f32)
    ff1 = np.asarray(ip['ff1_w'], f32)
    w['ff1bd'] = _bd((ff1 * ln2w[None, :]).T).astype(BF16)
    ff2 = np.asarray(ip['ff2_w'], f32).copy()
    ff2[0, :] = 0.0
    w['ff2bd'] = _bd(ff2.T).astype(BF16)
    ff2b = np.asarray(ip['ff2_b'], f32).copy()
    ff2b[0] = 0.0
    w['ff2b'] = np.concatenate([ff2b, ff2b]).reshape(128, 1)

    e_of_p = (np.arange(128) // 64) * 4 + ((np.arange(128) % 64) // 16)
    dots = np.zeros((128, 9, 72), f32)
    for o in range(9):
        dots[np.arange(128), o, o * 8 + e_of_p] = 1.0
    w['dotslhs'] = dots.reshape(128, 9 * 72).astype(BF16)
    # fused Z-sum + broadcast: zbc[p,n] = sum_q [e_of_p[p]==q%8] E[q,n]
    #                                   = sum_o E[o*8+e(p), n] = Z(head(p), n)
    zr = np.zeros((72, 128), f32)
    for q in range(72):
        zr[q, e_of_p == (q % 8)] = 1.0
    w['zrcpbl'] = zr.astype(BF16)
    eb = np.zeros((72, 9, 128), f32)
    for o in range(9):
        eb[o * 8 + e_of_p, o, np.arange(128)] = 1.0
    w['eblhs'] = eb.reshape(72, 9 * 128).astype(BF16)
    w['id128'] = np.eye(128, dtype=f32).astype(BF16)
    w['onesbd'] = _bd(np.ones((64, 64), f32)).astype(BF16)
    w['ones128f'] = np.ones((128, 1), f32)
    w['onesrowf'] = np.ones((1, 128), f32)
    swp = np.zeros((128, 128), f32)
    swp[0:64, 64:128] = np.eye(64)
    swp[64:128, 0:64] = np.eye(64)
    w['swapb'] = swp.astype(BF16)          # half-swap for bf16 halo
    w['swapf'] = swp.astype(f32)           # half-swap for fp32 halo
    return w


def _wspecs(dt):
    return {
        'convp0': ([128, 3 * 128], dt.bfloat16),
        'convp1': ([128, 3 * 128], dt.bfloat16),
        'convs': ([128, 3 * 128], dt.bfloat16),
        'convc': ([128, 128], dt.bfloat16),
        'fc1t': ([128, 64], dt.bfloat16), 'fc1rs_col': ([128, 1], dt.float32),
        'qkvbd': ([128, 384], dt.bfloat16),
        'outwbd': ([128, 128], dt.bfloat16), 'outb': ([128, 1], dt.float32),
        'ff1bd': ([128, 128], dt.bfloat16), 'ff2bd': ([128, 128], dt.bfloat16),
        'ff2b': ([128, 1], dt.float32),
        'dotslhs': ([128, 9 * 72], dt.bfloat16),
        'zrcpbl': ([72, 128], dt.bfloat16), 'eblhs': ([72, 9 * 128], dt.bfloat16),
        'id128': ([128, 128], dt.bfloat16), 'onesbd': ([128, 128], dt.bfloat16),
        'ones128f': ([128, 1], dt.float32), 'onesrowf': ([1, 128], dt.float32),
        'swapb': ([128, 128], dt.bfloat16), 'swapf': ([128, 128], dt.float32),
    }


def _build(reps=1):
    key = ('nc', reps)
    if key in _BUILD_CACHE:
        return _BUILD_CACHE[key]
    import concourse.bass as bass
    import concourse.bacc as bacc
    import concourse.tile as tile
    from concourse import mybir
    dt = mybir.dt
    ALU = mybir.AluOpType
    AF = mybir.ActivationFunctionType
    AX = mybir.AxisListType

    nc = bacc.Bacc("TRN2", target_bir_lowering=False)
    wspecs = _wspecs(dt)

    d_x0 = nc.dram_tensor('x0', [128, NPAD], dt.float32, kind='ExternalInput')
    d_mask = nc.dram_tensor('maskrep', [STEPS, 128, NH], dt.bfloat16, kind='ExternalInput')
    d_w = {k: nc.dram_tensor(k, list(s), d, kind='ExternalInput') for k, (s, d) in wspecs.items()}
    d_out = nc.dram_tensor('xout', [128, NH], dt.float32, kind='ExternalOutput')

    A = lambda h: h.ap()

    with tile.TileContext(nc) as tc:
        xpad = nc.alloc_sbuf_tensor('xpad', [128, NPAD], dt.float32)
        xb0 = nc.alloc_sbuf_tensor('xb0', [128, NPAD], dt.bfloat16)
        xb1 = nc.alloc_sbuf_tensor('xb1', [128, NPAD], dt.bfloat16)
        xc0 = nc.alloc_sbuf_tensor('xc0', [128, NPAD], dt.bfloat16)
        xc1 = nc.alloc_sbuf_tensor('xc1', [128, NPAD], dt.bfloat16)
        maskS = nc.alloc_sbuf_tensor('maskS', [128, STEPS * NH], dt.bfloat16)
        sw = {k: nc.alloc_sbuf_tensor('w_' + k, list(s), d) for k, (s, d) in wspecs.items()}
        hd = nc.alloc_sbuf_tensor('hd', [128, 2 * NH], dt.bfloat16)
        hdsq = nc.alloc_sbuf_tensor('hdsq', [128, 2048], dt.bfloat16)   # scratch
        accRS = nc.alloc_sbuf_tensor('accRS', [128, 8], dt.float32)
        accP = nc.alloc_sbuf_tensor('accP', [128, 2], dt.float32)
        tB = nc.alloc_sbuf_tensor('tB', [128, NH], dt.bfloat16)
        t2B = nc.alloc_sbuf_tensor('t2B', [128, NH], dt.bfloat16)
        uB = nc.alloc_sbuf_tensor('uB', [128, NH], dt.float32)
        varB = nc.alloc_sbuf_tensor('varB', [128, NH], dt.float32)
        invB = nc.alloc_sbuf_tensor('invB', [128, NH], dt.bfloat16)
        dnegB = nc.alloc_sbuf_tensor('dnegB', [128, NH], dt.bfloat16)
        ynegB = nc.alloc_sbuf_tensor('ynegB', [128, NH], dt.bfloat16)
        tmpB = nc.alloc_sbuf_tensor('tmpB', [128, NH], dt.float32)
        mi_b = nc.alloc_sbuf_tensor('mi_b', [128, NH], dt.bfloat16)
        qS = nc.alloc_sbuf_tensor('qS', [128, NH], dt.bfloat16)
        kpad = nc.alloc_sbuf_tensor('kpad', [128, NPAD], dt.bfloat16)
        vpadA = nc.alloc_sbuf_tensor('vpadA', [128, NPAD], dt.bfloat16)
        Ebuf = nc.alloc_sbuf_tensor('Ebuf', [72, NH], dt.bfloat16)
        rcpb = nc.alloc_sbuf_tensor('rcpb', [128, NH], dt.float32)
        pB = nc.alloc_sbuf_tensor('pB', [128, NH], dt.bfloat16)
        ebS = nc.alloc_sbuf_tensor('ebS', [128, NH], dt.bfloat16)
        oS = nc.alloc_sbuf_tensor('oS', [128, NH], dt.bfloat16)
        gB = nc.alloc_sbuf_tensor('gB', [128, NH], dt.bfloat16)
        tmpd = nc.alloc_sbuf_tensor('tmpd', [128, NH], dt.bfloat16)
        sc = nc.alloc_sbuf_tensor('scal', [1, 16], dt.float32)
        row2 = nc.alloc_sbuf_tensor('row2', [1, 2], dt.bfloat16)
        bc3 = nc.alloc_sbuf_tensor('bc3', [128, 2], dt.float32)
        corrc = nc.alloc_sbuf_tensor('corrc', [128, 1], dt.float32)
        epsb = nc.alloc_sbuf_tensor('epsb', [128, 1], dt.float32)
        eps2b = nc.alloc_sbuf_tensor('eps2b', [128, 1], dt.float32)

        v3 = lambda h: A(h).rearrange('p (r c) -> p r c', r=PR)
        cv3 = lambda h: v3(h)[:, 1:33, 2:66]
        r3 = lambda ap, cols: ap[:, cols].rearrange('p (r c) -> p r c', c=64)

        nc.sync.dma_start(out=A(xpad), in_=A(d_x0))
        for k in wspecs:
            nc.sync.dma_start(out=A(sw[k]), in_=A(d_w[k]))
        # masks last on the SP queue (first-step mask first) so weights
        # don't queue behind the big transfer
        mview = A(maskS).rearrange('p (s n) -> p s n', s=STEPS)
        dview = A(d_mask).rearrange('s p n -> p s n')
        nc.sync.dma_start(out=mview[:, 0:1, :], in_=dview[:, 0:1, :])
        # bulk of the masks via the ACT engine's DGE so step-0's xb shift
        # DMAs don't queue behind this 20KB/partition transfer on SP
        nc.scalar.dma_start(out=mview[:, 1:STEPS, :], in_=dview[:, 1:STEPS, :])
        nc.vector.memset(A(epsb), EPS)
        nc.vector.memset(A(eps2b), 4096.0 * EPS)
        nc.vector.memset(A(kpad), 0.0)
        nc.vector.memset(A(vpadA), 0.0)

        import contextlib
        stack = contextlib.ExitStack()
        p1k = stack.enter_context(tc.tile_pool(name='p1k', bufs=2, space='PSUM'))
        pdot = stack.enter_context(tc.tile_pool(name='pdot', bufs=1, space='PSUM'))
        pacc = stack.enter_context(tc.tile_pool(name='pacc', bufs=1, space='PSUM'))

        NT = NH // 512

        def stt(eng, out, in0, op0, scalar, op1, in1, accum_out=None):
            eng.scalar_tensor_tensor(out=out, in0=in0, scalar=scalar, in1=in1,
                                     op0=op0, op1=op1, accum_out=accum_out)

        def halo_swap(buf, lhs, evict=None):
            """refresh halo rows: half0 row33 <- half1 row1; half1 row0 <- half0 row32.
            One [128, 2, 68] block-swap matmul + per-quadrant evictions (the other
            two quadrants hold reflect rows / zero padding and must stay intact).
            The step-tail xpad halo evicts on ACT so the next step's xb copies
            don't queue behind it on DVE."""
            rhs = v3(buf)[:, 1:33:31, :]          # rows 1, 32 of both halves
            hp = p1k.tile([128, 512], dt.float32, tag='t1k')
            h3 = hp[:, 0:136].rearrange('p (r c) -> p r c', c=68)
            nc.tensor.matmul(h3, A(sw[lhs]), rhs, start=True, stop=True)
            if evict == 'act':
                nc.scalar.activation(v3(buf)[0:64, 33:34, :], h3[0:64, 0:1, :], AF.Copy)
                nc.scalar.activation(v3(buf)[64:128, 0:1, :], h3[64:128, 1:2, :], AF.Copy)
            else:
                nc.vector.tensor_copy(out=v3(buf)[0:64, 33:34, :], in_=h3[0:64, 0:1, :])
                nc.vector.tensor_copy(out=v3(buf)[64:128, 0:1, :], in_=h3[64:128, 1:2, :])

        for gstep in range(STEPS * reps):
            step = gstep % STEPS
            mstep = A(maskS)[:, step * NH:(step + 1) * NH]



            # bf16 shadows: xb0 = [half0; half0+1row], xb1 = [half1+1row; half1].
            # Split by row range so conv k=0 (rows 0:18) starts while the
            # second halves still copy.
            CUT = 18 * PITCH
            nc.vector.tensor_copy(out=A(xb0)[0:64, 0:CUT], in_=A(xpad)[0:64, 0:CUT])
            nc.gpsimd.tensor_copy(out=A(xb1)[64:128, 0:CUT], in_=A(xpad)[64:128, 0:CUT])
            nc.sync.dma_start(out=A(xb0)[64:128, 0:CUT - PITCH],
                              in_=A(xb0)[0:64, PITCH:CUT])
            nc.sync.dma_start(out=A(xb1)[0:64, 0:CUT - PITCH],
                              in_=A(xb1)[64:128, PITCH:CUT])
            nc.vector.tensor_copy(out=A(xb0)[0:64, CUT:NPAD], in_=A(xpad)[0:64, CUT:NPAD])
            nc.gpsimd.tensor_copy(out=A(xb1)[64:128, CUT:NPAD], in_=A(xpad)[64:128, CUT:NPAD])
            # xc stacks for the di=+1 column-shift pairing; SP-queue order is
            # chosen so each DMA lands just before its consumer group closes
            nc.sync.dma_start(out=A(xc0)[0:64, 0:CUT - PITCH],
                              in_=A(xb0)[64:128, 0:CUT - PITCH])
            nc.sync.dma_start(out=A(xc0)[64:128, 0:CUT - PITCH - 1],
                              in_=A(xb0)[64:128, 1:CUT - PITCH])
            nc.sync.dma_start(out=A(xb0)[64:128, CUT - PITCH:NPAD - PITCH],
                              in_=A(xb0)[0:64, CUT:NPAD])
            nc.sync.dma_start(out=A(xb1)[0:64, CUT - PITCH:NPAD - PITCH],
                              in_=A(xb1)[64:128, CUT:NPAD])
            nc.sync.dma_start(out=A(xc1)[0:64, 0:CUT - PITCH],
                              in_=A(xb1)[0:64, 0:CUT - PITCH])
            nc.sync.dma_start(out=A(xc1)[64:128, 0:CUT - PITCH - 1],
                              in_=A(xb1)[0:64, 1:CUT - PITCH])
            nc.sync.dma_start(out=A(xc0)[0:64, CUT - PITCH:NPAD - PITCH],
                              in_=A(xb0)[64:128, CUT - PITCH:NPAD - PITCH])
            nc.sync.dma_start(out=A(xc0)[64:128, CUT - PITCH - 1:NPAD - PITCH - 1],
                              in_=A(xb0)[64:128, CUT - PITCH:NPAD - PITCH])
            nc.sync.dma_start(out=A(xc1)[0:64, CUT - PITCH:NPAD - PITCH],
                              in_=A(xb1)[0:64, CUT - PITCH:NPAD - PITCH])
            nc.sync.dma_start(out=A(xc1)[64:128, CUT - PITCH - 1:NPAD - PITCH - 1],
                              in_=A(xb1)[0:64, CUT - PITCH:NPAD - PITCH])

            # fused conv+fc0 -> relu -> hd ; accum_out: sum(hd) per partition
            # singles (di=+1, 64-contraction) first, then row-paired matmuls
            for k in range(2):
                # all four conv PSUM tiles live at once (pdot/pacc are idle
                # here) so no matmul group ever waits on an hd eviction
                if k == 0:
                    hps = [p1k.tile([128, 1024], dt.float32, tag='t1k', name='hp00'),
                           pdot.tile([128, 1024], dt.float32, tag='pd', name='hp01')]
                else:
                    hps = [p1k.tile([128, 1024], dt.float32, tag='t1k', name='hp10'),
                           pacc.tile([128, 1024], dt.float32, tag='oacc', name='hp11')]
                for s in range(2):
                    xb = xb0 if s == 0 else xb1
                    spart = slice(0, 64) if s == 0 else slice(64, 128)
                    for t in range(2):
                        rs = 16 * k + 8 * t
                        rhs = v3(xb)[spart, 2 + rs:10 + rs, 3:67]
                        nc.tensor.matmul(hps[s][:, 512 * t:512 * t + 512],
                                         A(sw['convs'])[spart, 2 * 128:3 * 128],
                                         rhs, start=True, stop=False)
                for s in range(2):
                    xb = xb0 if s == 0 else xb1
                    pw = 'convp0' if s == 0 else 'convp1'
                    for t in range(2):
                        rs = 16 * k + 8 * t
                        for j in range(3):
                            rhs = v3(xb)[:, 0 + rs:8 + rs, 1 + j:65 + j]
                            nc.tensor.matmul(hps[s][:, 512 * t:512 * t + 512],
                                             A(sw[pw])[:, j * 128:(j + 1) * 128],
                                             rhs, start=False, stop=False)
                for s in range(2):
                    xc = xc0 if s == 0 else xc1
                    for t in range(2):
                        rs = 16 * k + 8 * t
                        rhs = v3(xc)[:, 1 + rs:9 + rs, 1:65]
                        nc.tensor.matmul(hps[s][:, 512 * t:512 * t + 512],
                                         A(sw['convc']), rhs, start=False, stop=True)
                for s in range(2):
                    hp = hps[s]
                    col = s * NH + 1024 * k
                    ci = 2 * s + k
                    nc.scalar.activation(A(hd)[:, col:col + 1024], hp[:], AF.Relu,
                                         accum_out=A(accRS)[:, ci:ci + 1])

                    hsl = slice(1024 * (ci % 2), 1024 * (ci % 2) + 1024)
                    if ci % 2 == 0:
                        nc.scalar.activation(A(hdsq)[:, hsl], A(hd)[:, col:col + 1024],
                                             AF.Square, accum_out=A(accRS)[:, 4 + ci:5 + ci])
                    else:
                        stt(nc.vector, A(hdsq)[:, hsl], A(hd)[:, col:col + 1024],
                            ALU.bypass, 0.0, ALU.mult, A(hd)[:, col:col + 1024],
                            accum_out=A(accRS)[:, 4 + ci:5 + ci])

            # fc1 matmuls first: they need only hd, not the stats, and
            # emitting them ahead keeps the PE from head-of-line blocking
            # behind the tiny stats matmuls
            dps = []
            for k in range(2):
                dp = p1k.tile([128, 1024], dt.float32, tag='t1k', name=f'dp{k}')
                for half in range(2):
                    for t in range(2):
                        colh = 1024 * k + 512 * t
                        osl = (slice(64 * half, 64 * half + 64), slice(512 * t, 512 * t + 512))
                        nc.tensor.matmul(dp[osl[0], osl[1]], A(sw['fc1t']),
                                         A(hd)[:, half * NH + colh:half * NH + colh + 512],
                                         start=True, stop=True)
                dps.append(dp)
            # prefetch the abs_rsqrt act table (for norm0/LN1) anchored on
            # the fresh dp0 PSUM tile -- an SBUF anchor can be WAR-hoisted
            # into the previous step's gelu region, thrashing the table
            nc.scalar.activation(A(sc)[:, 13:14], dps[0][0:1, 0:1],
                                 AF.Abs_reciprocal_sqrt)

            # norm0 global stats from accumulators
            nc.vector.tensor_reduce(A(accP).rearrange('p (a b) -> p a b', b=1),
                                    A(accRS).rearrange('p (a b) -> p a b', b=4),
                                    AX.X, ALU.add)
            sp = pacc.tile([1, 2], dt.float32, tag='oacc', name='sp')
            nc.tensor.matmul(sp[:], A(sw['ones128f']), A(accP), start=True, stop=True)
            mean, e2 = A(sc)[:, 2:3], A(sc)[:, 3:4]
            negvar = A(sc)[:, 4:5]
            istd, nm = A(sc)[:, 9:10], A(sc)[:, 10:11]
            NTOT = 1.0 / (2 * NH * 128)
            nc.vector.tensor_scalar_mul(A(sc)[:, 2:4], sp[:], NTOT)
            stt(nc.vector, negvar, mean, ALU.mult, mean, ALU.subtract, e2)
            nc.vector.tensor_scalar_mul(nm, mean, -1.0)
            nc.scalar.activation(istd, negvar, AF.Abs_reciprocal_sqrt,
                                 bias=A(epsb)[0:1, :], scale=-1.0)
            bcp = pacc.tile([128, 2], dt.float32, tag='oacc', name='bcp')
            nc.tensor.matmul(bcp[:], A(sw['onesrowf']), A(sc)[:, 9:11],
                             start=True, stop=True)
            # corr = -mu * rowsum(fc1) per out-channel (bcp read from PSUM)
            stt(nc.vector, A(corrc), A(sw['fc1rs_col']), ALU.bypass, 0.0,
                ALU.mult, bcp[:, 1:2])
            # mask * istd as one bf16 tensor (tensor_scalar, 4x mode)
            nc.vector.tensor_scalar(out=A(mi_b), in0=mstep, scalar1=bcp[:, 0:1],
                                    scalar2=None, op0=ALU.mult)

            # dx ; x += (dp + corr) * (mask * istd)
            for k in range(2):
                cs = slice(1024 * k, 1024 * k + 1024)
                stt(nc.vector, A(tmpB)[:, cs], dps[k][:], ALU.add, A(corrc),
                    ALU.mult, A(mi_b)[:, cs])
                rows = slice(1 + 16 * k, 1 + 16 * k + 16)
                nc.gpsimd.tensor_add(v3(xpad)[:, rows, 2:66], r3(A(tmpB), cs),
                                     v3(xpad)[:, rows, 2:66])

            def ln_prep(kk):
                # tB copy (DVE/Pool) and x^2 (ACT Square straight off xpad)
                # run in parallel -- removes the copy->square serial hop
                cs2 = slice(1024 * kk, 1024 * kk + 1024)
                rows2 = slice(1 + 16 * kk, 1 + 16 * kk + 16)
                if kk == 0:
                    nc.vector.tensor_copy(out=A(tB)[:, cs2], in_=v3(xpad)[:, rows2, 2:66])
                    nc.scalar.activation(r3(A(t2B), cs2),
                                         v3(xpad)[:, rows2, 2:66], AF.Square)
                else:
                    nc.gpsimd.tensor_copy(out=A(tB)[:, cs2], in_=v3(xpad)[:, rows2, 2:66])
                    nc.gpsimd.tensor_mul(A(t2B)[:, cs2], A(tB)[:, cs2], A(tB)[:, cs2])

            def ln_finish(kk, dst):
                cs2 = slice(1024 * kk, 1024 * kk + 1024)
                mu = p1k.tile([128, 1024], dt.float32, tag='t1k')
                sq = p1k.tile([128, 1024], dt.float32, tag='t1k')
                for tt in range(2):
                    nsl = slice(1024 * kk + 512 * tt, 1024 * kk + 512 * tt + 512)
                    osl = slice(512 * tt, 512 * tt + 512)
                    nc.tensor.matmul(mu[:, osl], A(sw['onesbd']),
                                     A(tB)[:, nsl], start=True, stop=True)
                    nc.tensor.matmul(sq[:, osl], A(sw['onesbd']),
                                     A(t2B)[:, nsl], start=True, stop=True)
                nc.scalar.activation(A(uB)[:, cs2], mu[:], AF.Square)
                stt(nc.vector, A(varB)[:, cs2], sq[:], ALU.mult, 64.0,
                    ALU.subtract, A(uB)[:, cs2])
                nc.scalar.activation(A(invB)[:, cs2], A(varB)[:, cs2],
                                     AF.Abs_reciprocal_sqrt, bias=A(eps2b))
                rows2 = slice(1 + 16 * kk, 1 + 16 * kk + 16)
                stt(nc.vector, r3(A(dnegB), cs2), v3(xpad)[:, rows2, 2:66],
                    ALU.mult, 64.0, ALU.subtract, mu[:].rearrange('p (r c) -> p r c', c=64))
                nc.vector.tensor_mul(A(dst)[:, cs2], A(dnegB)[:, cs2],
                                     A(invB)[:, cs2])

            def layernorm_to(dst):
                ln_prep(0)
                ln_prep(1)
                ln_finish(0, dst)
                ln_finish(1, dst)

            # LN1 + qkv (block-diag: both halves per matmul); 1024-col PSUM
            # tiles so each q/k/v group needs one ACT eviction instead of two
            layernorm_to(ynegB)
            pr3 = lambda ps: ps[:].rearrange('p (r c) -> p r c', c=64)
            vps = []
            for k in range(2):
                cs = slice(1024 * k, 1024 * k + 1024)
                rr = slice(1 + 16 * k, 1 + 16 * k + 16)
                # qp borrows the pdot pool (and vp0 the pacc pool) so all
                # qkv tiles are live at once and the matmuls run gapless
                qp = pdot.tile([128, 1024], dt.float32, tag='pd', name=f'qp{k}')
                kp = p1k.tile([128, 1024], dt.float32, tag='t1k')
                vp = (pacc.tile([128, 1024], dt.float32, tag='oacc', name='vp0')
                      if k == 0 else
                      p1k.tile([128, 1024], dt.float32, tag='t1k', name='vp1'))
                vps.append(vp)
                for t in range(2):
                    nsl = slice(1024 * k + 512 * t, 1024 * k + 512 * t + 512)
                    fs = slice(512 * t, 512 * t + 512)
                    nc.tensor.matmul(qp[:, fs], A(sw['qkvbd'])[:, 0:128],
                                     A(ynegB)[:, nsl], start=True, stop=True)
                    nc.tensor.matmul(kp[:, fs], A(sw['qkvbd'])[:, 128:256],
                                     A(ynegB)[:, nsl], start=True, stop=True)
                    nc.tensor.matmul(vp[:, fs], A(sw['qkvbd'])[:, 256:384],
                                     A(ynegB)[:, nsl], start=True, stop=True)
                nc.scalar.activation(A(qS)[:, cs], qp[:], AF.Copy)
                nc.scalar.activation(v3(kpad)[:, rr, 2:66], pr3(kp), AF.Copy)
            halo_swap(kpad, 'swapb')
            # prefetch the exp act table while PE accumulates the dots
            # (anchored on the last qkv eviction; the occasional WAR-hoist
            # lands it in an overlapped slot -- pinning it measured slower);
            # vpad evictions are deferred past dots(k=0)
            nc.scalar.activation(A(sc)[:, 14:15], A(qS)[0:1, 1024:1025], AF.Exp)

            # dots + exp + eb, with dots(k=1) software-pipelined against
            # eb(k=0) so PE/DVE/Pool/ACT all stay fed through the attention
            TAPORD = [3, 4, 5, 0, 1, 2, 6, 7, 8]
            pdk = {}

            def dots_tap(k, idx, split):
                o = TAPORD[idx]
                di, dj = o // 3 - 1, o % 3 - 1
                po = (idx % 2) * 1024
                qcs = slice(1024 * k, 1024 * k + 1024)
                if split:
                    # latency-critical k=0: halve each tap's product across
                    # DVE || Pool so the PE never waits a full 1024-col mul
                    for h in range(2):
                        rows = slice(1 + di + 16 * k + 8 * h,
                                     1 + di + 16 * k + 8 * h + 8)
                        teng = nc.vector if h == 0 else nc.gpsimd
                        teng.tensor_mul(r3(A(tmpd), slice(po + 512 * h, po + 512 * h + 512)),
                                        r3(A(qS), slice(1024 * k + 512 * h,
                                                        1024 * k + 512 * h + 512)),
                                        v3(kpad)[:, rows, 2 + dj:66 + dj])
                else:
                    rows = slice(1 + di + 16 * k, 1 + di + 16 * k + 16)
                    teng = nc.gpsimd if idx % 2 == 1 else nc.vector
                    teng.tensor_mul(r3(A(tmpd), slice(po, po + 1024)),
                                    r3(A(qS), qcs),
                                    v3(kpad)[:, rows, 2 + dj:66 + dj])
                for t in range(2):
                    fs = slice(512 * t, 512 * t + 512)
                    nc.tensor.matmul(pdk[k][:, fs],
                                     A(sw['dotslhs'])[:, o * 72:(o + 1) * 72],
                                     A(tmpd)[:, po + 512 * t:po + 512 * t + 512],
                                     start=(idx == 0), stop=(idx == 8))

            def eb_tap(k, o, op_ps, first, last, do_z):
                di, dj = o // 3 - 1, o % 3 - 1
                cs = slice(1024 * k, 1024 * k + 1024)
                ebp = p1k.tile([128, 1024], dt.float32, tag='t1k')
                for t in range(2):
                    fs = slice(512 * t, 512 * t + 512)
                    nc.tensor.matmul(ebp[:, fs], A(sw['eblhs'])[:, o * 128:(o + 1) * 128],
                                     A(Ebuf)[:, 1024 * k + 512 * t:1024 * k + 512 * t + 512],
                                     start=True, stop=True)
                po = (o % 2) * 1024
                pcs = slice(po, po + 1024)
                rows = slice(1 + di + 16 * k, 1 + di + 16 * k + 16)
                if o in (0, 2, 4, 6):
                    # DVE reads the PSUM broadcast directly
                    stt(nc.vector, r3(A(pB), pcs),
                        ebp[:].rearrange('p (r c) -> p r c', c=64),
                        ALU.bypass, 0.0, ALU.mult, v3(vpadA)[:, rows, 2 + dj:66 + dj])
                else:
                    # ACT evicts the broadcast to bf16; Pool multiplies in SBUF
                    nc.scalar.activation(A(ebS)[:, pcs], ebp[:], AF.Copy)
                    nc.gpsimd.tensor_mul(r3(A(pB), pcs), r3(A(ebS), pcs),
                                         v3(vpadA)[:, rows, 2 + dj:66 + dj])
                for t in range(2):
                    fs = slice(512 * t, 512 * t + 512)
                    nc.tensor.matmul(op_ps[:, fs], A(sw['id128']),
                                     A(pB)[:, po + 512 * t:po + 512 * t + 512],
                                     start=first, stop=last)
                if do_z:
                    # fused Z-sum + 1/Z broadcast: one matmul straight off
                    # Ebuf (zrcpbl sums the 9 taps AND broadcasts per head)
                    zbc = p1k.tile([128, 1024], dt.float32, tag='t1k')
                    for t in range(2):
                        fs = slice(512 * t, 512 * t + 512)
                        nc.tensor.matmul(zbc[:, fs], A(sw['zrcpbl']),
                                         A(Ebuf)[:, 1024 * k + 512 * t:1024 * k + 512 * t + 512],
                                         start=True, stop=True)
                    nc.vector.reciprocal_approx_fast(A(rcpb)[:, cs], zbc[:])
                return ebp

            pdk[0] = pdot.tile([72, 1024], dt.float32, tag='pd', name='pd0')
            for idx in range(9):
                dots_tap(0, idx, split=True)
            nc.scalar.activation(A(Ebuf)[:, 0:1024], pdk[0][:], AF.Exp)
            # vpad evictions land here, filling the ACT gap while the
            # interleave warms up; halo right after
            for k in range(2):
                rr = slice(1 + 16 * k, 1 + 16 * k + 16)
                nc.scalar.activation(v3(vpadA)[:, rr, 2:66], pr3(vps[k]), AF.Copy)
            halo_swap(vpadA, 'swapb')

            pdk[1] = pdot.tile([72, 1024], dt.float32, tag='pd', name='pd1')
            op0 = pacc.tile([128, 1024], dt.float32, tag='oacc')
            for idx in range(9):
                dots_tap(1, idx, split=False)
                # halo-independent center-row taps first (TAPORD), so the
                # freshly-swapped vpad halo is never on the eb start path
                eb_tap(0, TAPORD[idx], op0, idx == 0, idx == 8, idx == 1)
            nc.scalar.activation(A(Ebuf)[:, 1024:2048], pdk[1][:], AF.Exp)
            stt(nc.vector, A(oS)[:, 0:1024], op0[:],
                ALU.bypass, 0.0, ALU.mult, A(rcpb)[:, 0:1024])

            # out-proj k=0 + LN2 kk=0 prep run UNDER the eb(k=1) loop:
            # out-proj k0 borrows the pdot slot freed by the k=1 exp
            def outproj(k, pool):
                ap_ps = pool.tile([128, 1024], dt.float32, tag=pool is p1k and 't1k' or 'pd',
                                  name=f'ap{k}')
                for t in range(2):
                    nsl = slice(1024 * k + 512 * t, 1024 * k + 512 * t + 512)
                    nc.tensor.matmul(ap_ps[:, 512 * t:512 * t + 512],
                                     A(sw['outwbd']), A(oS)[:, nsl],
                                     start=True, stop=True)
                rows = slice(1 + 16 * k, 1 + 16 * k + 16)
                stt(nc.vector, v3(xpad)[:, rows, 2:66],
                    ap_ps[:].rearrange('p (r c) -> p r c', c=64),
                    ALU.add, A(sw['outb']), ALU.add, v3(xpad)[:, rows, 2:66])

            outproj(0, pdot)
            ln_prep(0)
            # prefetch the abs_rsqrt table for LN2 (anchored past the last
            # Exp via Ebuf's k=1 chunk)
            nc.scalar.activation(A(sc)[:, 13:14], A(Ebuf)[0:1, 1024:1025],
                                 AF.Abs_reciprocal_sqrt)

            op1 = pacc.tile([128, 1024], dt.float32, tag='oacc')
            for idx in range(9):
                eb_tap(1, TAPORD[idx], op1, idx == 0, idx == 8, idx == 1)
            ln_finish(0, ynegB)
            # ff1 k=0 matmuls hoisted into the out-proj(k1)/LN2-kk1 window
            # via the pdot slot (ap0 was freed by outproj(0)'s residual);
            # only the gelu EVICTIONS must wait for the table load below
            fp0 = pdot.tile([128, 1024], dt.float32, tag='pd', name='fp0')
            for t in range(2):
                fs = slice(512 * t, 512 * t + 512)
                nc.tensor.matmul(fp0[:, fs], A(sw['ff1bd']),
                                 A(ynegB)[:, fs], start=True, stop=True)
            stt(nc.vector, A(oS)[:, 1024:2048], op1[:],
                ALU.bypass, 0.0, ALU.mult, A(rcpb)[:, 1024:2048])

            outproj(1, p1k)
            ln_prep(1)
            ln_finish(1, ynegB)
            fp1 = p1k.tile([128, 1024], dt.float32, tag='t1k', name='fp1')
            for t in range(2):
                nsl = slice(1024 + 512 * t, 1024 + 512 * t + 512)
                nc.tensor.matmul(fp1[:, 512 * t:512 * t + 512], A(sw['ff1bd']),
                                 A(ynegB)[:, nsl], start=True, stop=True)
            # gelu-table prefetch: the scheduler WAR-hoists this read above
            # LN2's ynegB write on some steps (costing a reload) but the
            # early overlapped load still nets faster than a pinned one
            nc.scalar.activation(A(sc)[:, 15:16], A(ynegB)[0:1, 1024:1025], AF.Gelu)
            nc.scalar.activation(A(gB)[:, 0:1024], fp0[:], AF.Gelu)
            nc.scalar.activation(A(gB)[:, 1024:2048], fp1[:], AF.Gelu)
            for k in range(2):
                f2 = p1k.tile([128, 1024], dt.float32, tag='t1k')
                for t in range(2):
                    nsl = slice(1024 * k + 512 * t, 1024 * k + 512 * t + 512)
                    nc.tensor.matmul(f2[:, 512 * t:512 * t + 512],
                                     A(sw['ff2bd']), A(gB)[:, nsl],
                                     start=True, stop=True)
                rows = slice(1 + 16 * k, 1 + 16 * k + 16)
                f23 = f2[:].rearrange('p (r c) -> p r c', c=64)
                stt(nc.vector, v3(xpad)[:, rows, 2:66], f23,
                    ALU.add, A(sw['ff2b']), ALU.add, v3(xpad)[:, rows, 2:66])
                # border maintenance immediately per row-half: the k=0 piece
                # (incl. the row-0 reflect) finishes while ff2 k=1 still runs,
                # so the next conv's xb0 chunk-1 copy has no halo dependency
                if gstep < STEPS * reps - 1:
                    nc.gpsimd.tensor_copy(out=v3(xpad)[:, rows, 1:2],
                                          in_=v3(xpad)[:, rows, 3:4])
                    nc.vector.tensor_copy(out=v3(xpad)[:, rows, 66:67],
                                          in_=v3(xpad)[:, rows, 64:65])
                    if k == 0:
                        nc.scalar.copy(v3(xpad)[0:64, 0:1, :], v3(xpad)[0:64, 2:3, :])
                    else:
                        nc.scalar.copy(v3(xpad)[64:128, 33:34, :],
                                       v3(xpad)[64:128, 31:32, :])

            # input channel 0 (partitions 0/64) is frozen by zeroed weight
            # columns in fc1t/outwbd/ff2bd -- no restore pass needed

            if gstep < STEPS * reps - 1:
                halo_swap(xpad, 'swapf', evict='act')

        nc.sync.dma_start(out=A(d_out).rearrange('p (r c) -> p r c', c=64), in_=cv3(xpad))
        stack.close()

    nc.compile()
    _BUILD_CACHE[key] = nc
    return nc


def _make_in_maps(x, masks, w):
    in_maps = []
    for b in range(B):
        m = dict(w)
        m['x0'] = _pad_state(x[b])
        mk = masks[:, b, :, :, 0].astype(np.float32)
        mrep = np.zeros((STEPS, 128, NH), np.float32)
        for s in range(2):
            row = mk[:, 32 * s:32 * s + 32, :].reshape(STEPS, NH)
            mrep[:, 64 * s:64 * s + 64, :] = row[:, None, :]
        m['maskrep'] = mrep.astype(BF16)
        in_maps.append(m)
    return in_maps


def kernel(**inputs):
    from concourse.bass_utils import run_bass_kernel_spmd

    x = np.asarray(inputs['x'], np.float32)
    masks = np.asarray(inputs['masks'])
    nc = _build()
    w = _pack_weights(inputs)
    in_maps = _make_in_maps(x, masks, w)

    import os
    trace = bool(os.environ.get('BASS_TRACE_RUN'))
    res = run_bass_kernel_spmd(nc, in_maps, core_ids=list(range(B)), trace=trace)
    if trace:
        print('exec_time_ns:', res.exec_time_ns)
        if res.profile_json:
            print('profile_json:', res.profile_json)
    out = np.zeros((B, H, W, C), np.float32)
    for b in range(B):
        xo = np.asarray(res.results[b]['xout'], np.float32)
        for s in range(2):
            blk = xo[64 * s:64 * s + 64].reshape(64, RH, W)
            out[b, 32 * s:32 * s + 32] = np.transpose(blk, (1, 2, 0))
    return out

